# revision 6
# baseline (speedup 1.0000x reference)
"""Multi-head attention (B=2, S=2048, D=1024, H=16) on 8 TRN2 NeuronCores.

Sharding: batch (2) x head-groups (4 heads each) -> 8 cores. Each core
computes QKV projections for its 256 output dims (4 heads) over its batch
element, then full attention for those 4 heads, producing out[b, :, g*256:+256].

Device-side layout: activations are kept feature-major ("transposed",
[dim, seq]) so the contraction dim always sits on SBUF partitions:
  - qpT/kpT [256, 2048] via matmuls lhsT=WT chunk, rhs=xT chunk (fp32r)
  - scores-transposed S'[nk, nq] per 128-key chunk; exp on ACT (scale=1/8)
    straight out of PSUM into fp16 E chunks
  - AV: lhsT=[V|1] fp16 [128, 65], rhs=E chunk, accumulated over 16 chunks
    -> [65, 512] = [out.T | denom]
  - PE-transpose back to [nq, 65], normalize rows with reciprocal(denom)
"""

import numpy as np

B, S, D = 2, 2048, 1024
H, DH = 16, 64
NCORES = 8
HPC = H // (NCORES // B)  # heads per core = 4
DS = HPC * DH  # output dims per core = 256
KC = D // 128  # contraction chunks = 8
NQ = 4  # query chunks of 512
NK = S // 128  # key chunks = 16
SEQC = S // 128  # seq chunks of 128 = 16

_CACHE = {}


def _build():
    import concourse.bass as bass
    import concourse.tile as tile
    from concourse import bacc, mybir
    from contextlib import ExitStack

    f32 = mybir.dt.float32
    f32r = mybir.dt.float32r
    f16 = mybir.dt.float16

    nc = bacc.Bacc("TRN2", target_bir_lowering=False, debug=False)

    qT = nc.dram_tensor("qT", [D, S], f16, kind="ExternalInput").ap()
    kT = nc.dram_tensor("kT", [D, S], f16, kind="ExternalInput").ap()
    vT = nc.dram_tensor("vT", [D, S], f16, kind="ExternalInput").ap()
    wqT = nc.dram_tensor("wqT", [D, DS], f16, kind="ExternalInput").ap()
    wkT = nc.dram_tensor("wkT", [D, DS], f16, kind="ExternalInput").ap()
    wvT = nc.dram_tensor("wvT", [D, DS], f16, kind="ExternalInput").ap()
    bq = nc.dram_tensor("bq", [DS], f32, kind="ExternalInput").ap()
    bk = nc.dram_tensor("bk", [DS], f32, kind="ExternalInput").ap()
    bv = nc.dram_tensor("bv", [DS], f32, kind="ExternalInput").ap()
    ident = nc.dram_tensor("ident", [128, 128], f16, kind="ExternalInput").ap()
    identf = nc.dram_tensor("identf", [128, 128], f32, kind="ExternalInput").ap()
    out = nc.dram_tensor("out", [S, DS], f32, kind="ExternalOutput").ap()

    ins_T = [qT, kT, vT]
    ins_W = [wqT, wkT, wvT]
    ins_B = [bq, bk, bv]

    with tile.TileContext(nc, trace_sim=False) as tc, ExitStack() as ctx:
        Exp = mybir.ActivationFunctionType.Exp

        const_pool = ctx.enter_context(tc.tile_pool(name="const", bufs=1))
        # weights: [128, KC*DS] (chunk k8 at free offset k8*DS)
        w_sb = []
        for t in range(3):
            w = const_pool.tile([128, KC * DS], f16, tag=f"w{t}", name=f"w{t}")
            nc.sync.dma_start(
                w[:].rearrange("p (kc m) -> p kc m", kc=KC),
                ins_W[t].rearrange("(kc p) m -> p kc m", p=128),
            )
            w_sb.append(w)
        b_sb = []
        for t in range(3):
            bt = const_pool.tile([128, 2], f32, tag=f"b{t}", name=f"b{t}")
            nc.sync.dma_start(bt[:], ins_B[t].rearrange("(c p) -> p c", p=128))
            b_sb.append(bt)
        id_sb = const_pool.tile([128, 128], f16, tag="ident")
        nc.sync.dma_start(id_sb[:], ident[:])
        idf_sb = const_pool.tile([65, 65], f32, tag="identf")
        nc.sync.dma_start(idf_sb[:], identf[0:65, 0:65])
        # warm the exp table while projections run
        scratch = const_pool.tile([128, 1], f32, tag="scratch")
        nc.scalar.activation(scratch[:], b_sb[0][:, 0:1], Exp)

        # persistent activations
        proj_pool = ctx.enter_context(tc.tile_pool(name="proj", bufs=1))
        qpT = [proj_pool.tile([128, S], f16, tag=f"qpT{m}", name=f"qpT{m}") for m in range(2)]
        kpT = [proj_pool.tile([128, S], f16, tag=f"kpT{m}", name=f"kpT{m}") for m in range(2)]
        # per head: [128, SEQC*65] fp16, col 64 of each 65-block is ones
        vpe = [proj_pool.tile([128, SEQC * 65], f16, tag=f"vpe{h}", name=f"vpe{h}") for h in range(HPC)]
        for h in range(HPC):
            nc.vector.memset(vpe[h][:, 64 :: 65], 1.0)

        out_sb = proj_pool.tile([128, SEQC * DS], f32, tag="out_sb")

        # ---------------- projections ----------------
        with (
            tc.tile_pool(name="pp", bufs=4, space="PSUM") as pp,
            tc.tile_pool(name="ptr", bufs=2, space="PSUM") as ptr,
            tc.tile_pool(name="rhs", bufs=4) as rhs_pool,
            tc.tile_pool(name="vtmp", bufs=3) as vtmp_pool,
        ):
            for t in range(3):
                for n in range(NQ):  # seq chunks of 512
                    ptiles = [pp.tile([128, 512], f32, tag="pp", name="pp") for _ in range(2)]
                    for k8 in range(KC):
                        rt = rhs_pool.tile([128, 512], f16, tag="rhs")
                        nc.sync.dma_start(
                            rt[:],
                            ins_T[t][k8 * 128 : (k8 + 1) * 128, n * 512 : (n + 1) * 512],
                        )
                        for m in range(2):
                            nc.tensor.matmul(
                                ptiles[m][:],
                                w_sb[t][:, k8 * DS + m * 128 : k8 * DS + (m + 1) * 128],
                                rt[:],
                                start=(k8 == 0),
                                stop=(k8 == KC - 1),
                            )
                    for m in range(2):
                        if t < 2:
                            dst = qpT[m] if t == 0 else kpT[m]
                            nc.vector.tensor_scalar_add(
                                dst[:, n * 512 : (n + 1) * 512],
                                ptiles[m][:],
                                b_sb[t][:, m : m + 1],
                            )
                        else:
                            vt = vtmp_pool.tile([128, 512], f16, tag="vtmp")
                            nc.vector.tensor_scalar_add(
                                vt[:], ptiles[m][:], b_sb[t][:, m : m + 1]
                            )
                            # transpose [128 dims, 128 seq] blocks -> vpe
                            for s4 in range(4):
                                c = n * 4 + s4  # global seq chunk
                                tp = ptr.tile([128, 128], f16, tag="ptr")
                                nc.tensor.transpose(
                                    tp[:],
                                    vt[:, s4 * 128 : (s4 + 1) * 128],
                                    id_sb[:],
                                )
                                for hh in range(2):
                                    h = m * 2 + hh
                                    nc.vector.tensor_copy(
                                        vpe[h][:, c * 65 : c * 65 + 64],
                                        tp[:, hh * 64 : hh * 64 + 64],
                                    )

        # ---------------- attention ----------------
        # Per chunk c: scores (4 MMs) -> exp -> AV of chunk c-1 (4 MMs, lag 1
        # so PE never waits on the current exp). All 4 AV accumulators stay
        # live across the chunk loop; out-transpose tiles share their slots.
        with (
            tc.tile_pool(name="sc", bufs=1, space="PSUM") as sc_pool,
            tc.tile_pool(name="av", bufs=4, space="PSUM") as av_pool,
            tc.tile_pool(name="epool", bufs=6) as e_pool,
            tc.tile_pool(name="avsb", bufs=2) as avsb_pool,
            tc.tile_pool(name="rcp", bufs=4) as rcp_pool,
        ):
            for h in range(HPC):
                m, off = h // 2, (h % 2) * 64

                avs = [av_pool.tile([65, 512], f32, tag="av", name=f"av{h}_{j}") for j in range(NQ)]

                def av_chunk(c, ec):
                    for j in range(NQ):
                        nc.tensor.matmul(
                            avs[j][:],
                            vpe[h][:, c * 65 : (c + 1) * 65],
                            ec[:, j * 512 : (j + 1) * 512],
                            start=(c == 0),
                            stop=(c == NK - 1),
                        )

                prev = None  # (c, e-chunk) pending AV
                for c in range(NK):
                    sc = sc_pool.tile([128, S], f32, tag="sc")
                    for j in range(NQ):
                        nc.tensor.matmul(
                            sc[:, j * 512 : (j + 1) * 512],
                            kpT[m][off : off + 64, c * 128 : (c + 1) * 128],
                            qpT[m][off : off + 64, j * 512 : (j + 1) * 512],
                            start=True,
                            stop=True,
                        )
                    ec = e_pool.tile([128, S], f16, tag="e")
                    nc.scalar.activation(ec[:], sc[:], Exp, scale=0.125)
                    if prev is not None:
                        av_chunk(*prev)
                    prev = (c, ec)
                av_chunk(*prev)

                avsb = avsb_pool.tile([65, S], f32, tag="avsb")
                for j in range(NQ):
                    nc.vector.tensor_copy(avsb[:, j * 512 : (j + 1) * 512], avs[j][:])
                # transpose back + normalize
                for s in range(SEQC):
                    tp = av_pool.tile([128, 65], f32, tag="av", name=f"tr{h}_{s}")
                    nc.tensor.transpose(
                        tp[:],
                        avsb[:, s * 128 : (s + 1) * 128],
                        idf_sb[:],
                    )
                    rc = rcp_pool.tile([128, 1], f32, tag="rcp")
                    nc.vector.reciprocal(rc[:], tp[:, 64:65])
                    nc.vector.tensor_scalar_mul(
                        out_sb[:, s * DS + h * 64 : s * DS + (h + 1) * 64],
                        tp[:, 0:64],
                        rc[:, 0:1],
                    )
                # per-head output store
                nc.sync.dma_start(
                    out[:, h * 64 : (h + 1) * 64].rearrange("(c p) d -> p c d", p=128),
                    out_sb[:]
                    .rearrange("p (c hd) -> p c hd", c=SEQC)[
                        :, :, h * 64 : (h + 1) * 64
                    ],
                )

    nc.compile()
    return nc


def _get_nc():
    if "nc" not in _CACHE:
        _CACHE["nc"] = _build()
    return _CACHE["nc"]


def _in_maps(q, k, v, Wq, bq, Wk, bk, Wv, bv):
    q, k, v = (np.asarray(x, np.float32) for x in (q, k, v))
    Wq, Wk, Wv = (np.asarray(x, np.float32) for x in (Wq, Wk, Wv))
    bq, bk, bv = (np.asarray(x, np.float32) for x in (bq, bk, bv))
    ident = np.eye(128, dtype=np.float32)
    maps = []
    for c in range(NCORES):
        b, g = divmod(c, NCORES // B)
        sl = slice(g * DS, (g + 1) * DS)
        maps.append(
            {
                "qT": np.ascontiguousarray(q[b].T).astype(np.float16),
                "kT": np.ascontiguousarray(k[b].T).astype(np.float16),
                "vT": np.ascontiguousarray(v[b].T).astype(np.float16),
                "wqT": np.ascontiguousarray(Wq[sl].T).astype(np.float16),
                "wkT": np.ascontiguousarray(Wk[sl].T).astype(np.float16),
                "wvT": np.ascontiguousarray(Wv[sl].T).astype(np.float16),
                "bq": np.ascontiguousarray(bq[sl]),
                "bk": np.ascontiguousarray(bk[sl]),
                "bv": np.ascontiguousarray(bv[sl]),
                "ident": ident.astype(np.float16),
                "identf": ident,
            }
        )
    return maps


def _assemble(results):
    out = np.empty((B, S, D), np.float32)
    for c in range(NCORES):
        b, g = divmod(c, NCORES // B)
        out[b, :, g * DS : (g + 1) * DS] = results[c]["out"]
    return out


def kernel(q, k, v, Wq, bq, Wk, bk, Wv, bv):
    from concourse.bass_utils import run_bass_kernel_spmd

    nc = _get_nc()
    maps = _in_maps(q, k, v, Wq, bq, Wk, bk, Wv, bv)
    res = run_bass_kernel_spmd(nc, maps, list(range(NCORES)))
    return _assemble(res.results)


def run_traced(q, k, v, Wq, bq, Wk, bk, Wv, bv, trace_cores=None):
    """Like kernel() but with NTFF profiling; returns (out, exec_time_ns, results)."""
    import sys, types

    if "antenv.axon_hooks" not in sys.modules:
        mod = types.ModuleType("antenv.axon_hooks")
        _hook = [None]
        mod.set_axon_ntff_profile_hook = lambda h: _hook.__setitem__(0, h)
        mod.get_axon_ntff_profile_hook = lambda: _hook[0]
        sys.modules["antenv.axon_hooks"] = mod
        import antenv

        antenv.axon_hooks = mod
        try:
            from trn_agent_boot.trn_boot import _ntff_profile_via_ctypes

            mod.set_axon_ntff_profile_hook(
                _ntff_profile_via_ctypes("/opt/axon/libaxon_pjrt.so")
            )
        except Exception as e:
            print("ntff hook setup failed:", e)
    import concourse.bass_utils as bass_utils

    bass_utils.upload_artifacts = lambda tmpdir: tmpdir

    nc = _get_nc()
    maps = _in_maps(q, k, v, Wq, bq, Wk, bk, Wv, bv)
    res = bass_utils.run_bass_kernel_spmd(
        nc, maps, list(range(NCORES)), trace=True, trace_cores=trace_cores
    )
    return _assemble(res.results), res.exec_time_ns, res


# revision 7
# speedup vs baseline: 1.1523x; 1.1523x over previous
"""Multi-head attention (B=2, S=2048, D=1024, H=16) on 8 TRN2 NeuronCores.

Sharding: batch (2) x head-groups (4 heads each) -> 8 cores. Each core
computes QKV projections for its 256 output dims (4 heads) over its batch
element, then full attention for those 4 heads, producing out[b, :, g*256:+256].

Device-side layout: activations are kept feature-major ("transposed",
[dim, seq]) so the contraction dim always sits on SBUF partitions:
  - qpT/kpT [256, 2048] via matmuls lhsT=WT chunk, rhs=xT chunk (fp32r)
  - scores-transposed S'[nk, nq] per 128-key chunk; exp on ACT (scale=1/8)
    straight out of PSUM into fp16 E chunks
  - AV: lhsT=[V|1] fp16 [128, 65], rhs=E chunk, accumulated over 16 chunks
    -> [65, 512] = [out.T | denom]
  - PE-transpose back to [nq, 65], normalize rows with reciprocal(denom)
"""

import numpy as np
import ml_dtypes

B, S, D = 2, 2048, 1024
H, DH = 16, 64
NCORES = 8
HPC = H // (NCORES // B)  # heads per core = 4
DS = HPC * DH  # output dims per core = 256
KC = D // 128  # contraction chunks = 8
NQ = 4  # query chunks of 512
NK = S // 128  # key chunks = 16
SEQC = S // 128  # seq chunks of 128 = 16

_CACHE = {}


def _build():
    import concourse.bass as bass
    import concourse.tile as tile
    from concourse import bacc, mybir
    from contextlib import ExitStack

    f32 = mybir.dt.float32
    f32r = mybir.dt.float32r
    f16 = mybir.dt.bfloat16

    nc = bacc.Bacc("TRN2", target_bir_lowering=False, debug=False)

    qT = nc.dram_tensor("qT", [D, S], f16, kind="ExternalInput").ap()
    kT = nc.dram_tensor("kT", [D, S], f16, kind="ExternalInput").ap()
    vT = nc.dram_tensor("vT", [D, S], f16, kind="ExternalInput").ap()
    wqT = nc.dram_tensor("wqT", [D, DS], f16, kind="ExternalInput").ap()
    wkT = nc.dram_tensor("wkT", [D, DS], f16, kind="ExternalInput").ap()
    wvT = nc.dram_tensor("wvT", [D, DS], f16, kind="ExternalInput").ap()
    bq = nc.dram_tensor("bq", [DS], f32, kind="ExternalInput").ap()
    bk = nc.dram_tensor("bk", [DS], f32, kind="ExternalInput").ap()
    bv = nc.dram_tensor("bv", [DS], f32, kind="ExternalInput").ap()
    ident = nc.dram_tensor("ident", [128, 128], f16, kind="ExternalInput").ap()
    identf = nc.dram_tensor("identf", [128, 128], f32, kind="ExternalInput").ap()
    out = nc.dram_tensor("out", [S, DS], f32, kind="ExternalOutput").ap()

    ins_T = [qT, kT, vT]
    ins_W = [wqT, wkT, wvT]
    ins_B = [bq, bk, bv]

    with tile.TileContext(nc, trace_sim=False) as tc, ExitStack() as ctx:
        Exp = mybir.ActivationFunctionType.Exp

        const_pool = ctx.enter_context(tc.tile_pool(name="const", bufs=1))
        # weights: [128, KC*DS] (chunk k8 at free offset k8*DS)
        w_sb = []
        for t in range(3):
            w = const_pool.tile([128, KC * DS], f16, tag=f"w{t}", name=f"w{t}")
            nc.sync.dma_start(
                w[:].rearrange("p (kc m) -> p kc m", kc=KC),
                ins_W[t].rearrange("(kc p) m -> p kc m", p=128),
            )
            w_sb.append(w)
        b_sb = []
        for t in range(3):
            bt = const_pool.tile([128, 2], f32, tag=f"b{t}", name=f"b{t}")
            nc.sync.dma_start(bt[:], ins_B[t].rearrange("(c p) -> p c", p=128))
            b_sb.append(bt)
        id_sb = const_pool.tile([128, 128], f16, tag="ident")
        nc.sync.dma_start(id_sb[:], ident[:])
        idf_sb = const_pool.tile([65, 65], f32, tag="identf")
        nc.sync.dma_start(idf_sb[:], identf[0:65, 0:65])
        # warm the exp table while projections run
        scratch = const_pool.tile([128, 1], f32, tag="scratch")
        nc.scalar.activation(scratch[:], b_sb[0][:, 0:1], Exp)

        # persistent activations
        proj_pool = ctx.enter_context(tc.tile_pool(name="proj", bufs=1))
        qpT = [proj_pool.tile([128, S], f16, tag=f"qpT{m}", name=f"qpT{m}") for m in range(2)]
        kpT = [proj_pool.tile([128, S], f16, tag=f"kpT{m}", name=f"kpT{m}") for m in range(2)]
        # per head: [128, SEQC*65] fp16, col 64 of each 65-block is ones
        vpe = [proj_pool.tile([128, SEQC * 65], f16, tag=f"vpe{h}", name=f"vpe{h}") for h in range(HPC)]
        for h in range(HPC):
            nc.vector.memset(vpe[h][:, 64 :: 65], 1.0)

        out_sb = proj_pool.tile([128, SEQC * DS], f32, tag="out_sb")

        # ---------------- projections ----------------
        with (
            tc.tile_pool(name="pp", bufs=4, space="PSUM") as pp,
            tc.tile_pool(name="ptr", bufs=2, space="PSUM") as ptr,
            tc.tile_pool(name="rhs", bufs=9) as rhs_pool,
            tc.tile_pool(name="vtmp", bufs=3) as vtmp_pool,
        ):
            for t in range(3):
                rts = []
                for k8 in range(KC):
                    rt = rhs_pool.tile([128, S], f16, tag="rhs", name=f"rt{t}_{k8}")
                    nc.sync.dma_start(rt[:], ins_T[t][k8 * 128 : (k8 + 1) * 128, :])
                    rts.append(rt)
                for n in range(NQ):  # seq chunks of 512
                    ptiles = [pp.tile([128, 512], f32, tag="pp", name="pp") for _ in range(2)]
                    for k8 in range(KC):
                        for m in range(2):
                            nc.tensor.matmul(
                                ptiles[m][:],
                                w_sb[t][:, k8 * DS + m * 128 : k8 * DS + (m + 1) * 128],
                                rts[k8][:, n * 512 : (n + 1) * 512],
                                start=(k8 == 0),
                                stop=(k8 == KC - 1),
                            )
                    for m in range(2):
                        if t < 2:
                            dst = qpT[m] if t == 0 else kpT[m]
                            nc.vector.tensor_scalar_add(
                                dst[:, n * 512 : (n + 1) * 512],
                                ptiles[m][:],
                                b_sb[t][:, m : m + 1],
                            )
                        else:
                            vt = vtmp_pool.tile([128, 512], f16, tag="vtmp")
                            nc.vector.tensor_scalar_add(
                                vt[:], ptiles[m][:], b_sb[t][:, m : m + 1]
                            )
                            # transpose [128 dims, 128 seq] blocks -> vpe
                            for s4 in range(4):
                                c = n * 4 + s4  # global seq chunk
                                tp = ptr.tile([128, 128], f16, tag="ptr")
                                nc.tensor.transpose(
                                    tp[:],
                                    vt[:, s4 * 128 : (s4 + 1) * 128],
                                    id_sb[:],
                                )
                                for hh in range(2):
                                    h = m * 2 + hh
                                    nc.vector.tensor_copy(
                                        vpe[h][:, c * 65 : c * 65 + 64],
                                        tp[:, hh * 64 : hh * 64 + 64],
                                    )

        # ---------------- attention ----------------
        # Per chunk c: scores (4 MMs) -> exp -> AV of chunk c-1 (4 MMs, lag 1
        # so PE never waits on the current exp). All 4 AV accumulators stay
        # live across the chunk loop; out-transpose tiles share their slots.
        with (
            tc.tile_pool(name="sc", bufs=1, space="PSUM") as sc_pool,
            tc.tile_pool(name="av", bufs=4, space="PSUM") as av_pool,
            tc.tile_pool(name="epool", bufs=6) as e_pool,
            tc.tile_pool(name="avsb", bufs=2) as avsb_pool,
            tc.tile_pool(name="rcp", bufs=4) as rcp_pool,
        ):
            for h in range(HPC):
                m, off = h // 2, (h % 2) * 64

                avs = [av_pool.tile([65, 512], f32, tag="av", name=f"av{h}_{j}") for j in range(NQ)]

                def av_chunk(c, ec):
                    for j in range(NQ):
                        nc.tensor.matmul(
                            avs[j][:],
                            vpe[h][:, c * 65 : (c + 1) * 65],
                            ec[:, j * 512 : (j + 1) * 512],
                            start=(c == 0),
                            stop=(c == NK - 1),
                        )

                prev = None  # (c, e-chunk) pending AV
                for c in range(NK):
                    sc = sc_pool.tile([128, S], f32, tag="sc")
                    for j in range(NQ):
                        nc.tensor.matmul(
                            sc[:, j * 512 : (j + 1) * 512],
                            kpT[m][off : off + 64, c * 128 : (c + 1) * 128],
                            qpT[m][off : off + 64, j * 512 : (j + 1) * 512],
                            start=True,
                            stop=True,
                        )
                    ec = e_pool.tile([128, S], f16, tag="e")
                    nc.scalar.activation(ec[:], sc[:], Exp, scale=0.125)
                    if prev is not None:
                        av_chunk(*prev)
                    prev = (c, ec)
                av_chunk(*prev)

                avsb = avsb_pool.tile([65, S], f32, tag="avsb")
                for j in range(NQ):
                    nc.vector.tensor_copy(avsb[:, j * 512 : (j + 1) * 512], avs[j][:])
                # transpose back + normalize
                for s in range(SEQC):
                    tp = av_pool.tile([128, 65], f32, tag="av", name=f"tr{h}_{s}")
                    nc.tensor.transpose(
                        tp[:],
                        avsb[:, s * 128 : (s + 1) * 128],
                        idf_sb[:],
                    )
                    rc = rcp_pool.tile([128, 1], f32, tag="rcp")
                    nc.vector.reciprocal(rc[:], tp[:, 64:65])
                    nc.vector.tensor_scalar_mul(
                        out_sb[:, s * DS + h * 64 : s * DS + (h + 1) * 64],
                        tp[:, 0:64],
                        rc[:, 0:1],
                    )
                # per-head output store
                nc.sync.dma_start(
                    out[:, h * 64 : (h + 1) * 64].rearrange("(c p) d -> p c d", p=128),
                    out_sb[:]
                    .rearrange("p (c hd) -> p c hd", c=SEQC)[
                        :, :, h * 64 : (h + 1) * 64
                    ],
                )

    nc.compile()
    return nc


def _get_nc():
    if "nc" not in _CACHE:
        _CACHE["nc"] = _build()
    return _CACHE["nc"]


def _in_maps(q, k, v, Wq, bq, Wk, bk, Wv, bv):
    q, k, v = (np.asarray(x, np.float32) for x in (q, k, v))
    Wq, Wk, Wv = (np.asarray(x, np.float32) for x in (Wq, Wk, Wv))
    bq, bk, bv = (np.asarray(x, np.float32) for x in (bq, bk, bv))
    ident = np.eye(128, dtype=np.float32)
    maps = []
    for c in range(NCORES):
        b, g = divmod(c, NCORES // B)
        sl = slice(g * DS, (g + 1) * DS)
        maps.append(
            {
                "qT": np.ascontiguousarray(q[b].T).astype(ml_dtypes.bfloat16),
                "kT": np.ascontiguousarray(k[b].T).astype(ml_dtypes.bfloat16),
                "vT": np.ascontiguousarray(v[b].T).astype(ml_dtypes.bfloat16),
                "wqT": np.ascontiguousarray(Wq[sl].T).astype(ml_dtypes.bfloat16),
                "wkT": np.ascontiguousarray(Wk[sl].T).astype(ml_dtypes.bfloat16),
                "wvT": np.ascontiguousarray(Wv[sl].T).astype(ml_dtypes.bfloat16),
                "bq": np.ascontiguousarray(bq[sl]),
                "bk": np.ascontiguousarray(bk[sl]),
                "bv": np.ascontiguousarray(bv[sl]),
                "ident": ident.astype(ml_dtypes.bfloat16),
                "identf": ident,
            }
        )
    return maps


def _assemble(results):
    out = np.empty((B, S, D), np.float32)
    for c in range(NCORES):
        b, g = divmod(c, NCORES // B)
        out[b, :, g * DS : (g + 1) * DS] = results[c]["out"]
    return out


def kernel(q, k, v, Wq, bq, Wk, bk, Wv, bv):
    from concourse.bass_utils import run_bass_kernel_spmd

    nc = _get_nc()
    maps = _in_maps(q, k, v, Wq, bq, Wk, bk, Wv, bv)
    res = run_bass_kernel_spmd(nc, maps, list(range(NCORES)))
    return _assemble(res.results)


def run_traced(q, k, v, Wq, bq, Wk, bk, Wv, bv, trace_cores=None):
    """Like kernel() but with NTFF profiling; returns (out, exec_time_ns, results)."""
    import sys, types

    if "antenv.axon_hooks" not in sys.modules:
        mod = types.ModuleType("antenv.axon_hooks")
        _hook = [None]
        mod.set_axon_ntff_profile_hook = lambda h: _hook.__setitem__(0, h)
        mod.get_axon_ntff_profile_hook = lambda: _hook[0]
        sys.modules["antenv.axon_hooks"] = mod
        import antenv

        antenv.axon_hooks = mod
        try:
            from trn_agent_boot.trn_boot import _ntff_profile_via_ctypes

            mod.set_axon_ntff_profile_hook(
                _ntff_profile_via_ctypes("/opt/axon/libaxon_pjrt.so")
            )
        except Exception as e:
            print("ntff hook setup failed:", e)
    import concourse.bass_utils as bass_utils

    bass_utils.upload_artifacts = lambda tmpdir: tmpdir

    nc = _get_nc()
    maps = _in_maps(q, k, v, Wq, bq, Wk, bk, Wv, bv)
    res = bass_utils.run_bass_kernel_spmd(
        nc, maps, list(range(NCORES)), trace=True, trace_cores=trace_cores
    )
    return _assemble(res.results), res.exec_time_ns, res


# revision 8
# speedup vs baseline: 1.5607x; 1.3544x over previous
"""Multi-head attention (B=2, S=2048, D=1024, H=16) on 8 TRN2 NeuronCores.

Sharding: batch (2) x head-groups (4 heads each) -> 8 cores. Each core
computes QKV projections for its 256 output dims (4 heads) over its batch
element, then full attention for those 4 heads, producing out[b, :, g*256:+256].

Device-side layout: activations are kept feature-major ("transposed",
[dim, seq]) so the contraction dim always sits on SBUF partitions:
  - qpT/kpT [256, 2048] via matmuls lhsT=WT chunk, rhs=xT chunk (fp32r)
  - scores-transposed S'[nk, nq] per 128-key chunk; exp on ACT (scale=1/8)
    straight out of PSUM into fp16 E chunks
  - AV: lhsT=[V|1] fp16 [128, 65], rhs=E chunk, accumulated over 16 chunks
    -> [65, 512] = [out.T | denom]
  - PE-transpose back to [nq, 65], normalize rows with reciprocal(denom)
"""

import numpy as np
import ml_dtypes

B, S, D = 2, 2048, 1024
H, DH = 16, 64
NCORES = 8
HPC = H // (NCORES // B)  # heads per core = 4
DS = HPC * DH  # output dims per core = 256
KC = D // 128  # contraction chunks = 8
NQ = 4  # query chunks of 512
NK = S // 128  # key chunks = 16
SEQC = S // 128  # seq chunks of 128 = 16

_CACHE = {}


def _build():
    import concourse.bass as bass
    import concourse.tile as tile
    from concourse import bacc, mybir
    from contextlib import ExitStack

    f32 = mybir.dt.float32
    f32r = mybir.dt.float32r
    f16 = mybir.dt.bfloat16

    nc = bacc.Bacc("TRN2", target_bir_lowering=False, debug=False)

    qT = nc.dram_tensor("qT", [D, S], f16, kind="ExternalInput").ap()
    kT = nc.dram_tensor("kT", [D, S], f16, kind="ExternalInput").ap()
    vT = nc.dram_tensor("vT", [D, S], f16, kind="ExternalInput").ap()
    wqT = nc.dram_tensor("wqT", [D, DS], f16, kind="ExternalInput").ap()
    wkT = nc.dram_tensor("wkT", [D, DS], f16, kind="ExternalInput").ap()
    wvT = nc.dram_tensor("wvT", [D, DS], f16, kind="ExternalInput").ap()
    bq = nc.dram_tensor("bq", [DS], f32, kind="ExternalInput").ap()
    bk = nc.dram_tensor("bk", [DS], f32, kind="ExternalInput").ap()
    bv = nc.dram_tensor("bv", [DS], f32, kind="ExternalInput").ap()
    ident = nc.dram_tensor("ident", [128, 128], f16, kind="ExternalInput").ap()
    identf = nc.dram_tensor("identf", [128, 128], f32, kind="ExternalInput").ap()
    out = nc.dram_tensor("out", [S, DS], f32, kind="ExternalOutput").ap()

    ins_T = [qT, kT, vT]
    ins_W = [wqT, wkT, wvT]
    ins_B = [bq, bk, bv]

    with tile.TileContext(nc, trace_sim=False) as tc, ExitStack() as ctx:
        Exp = mybir.ActivationFunctionType.Exp

        const_pool = ctx.enter_context(tc.tile_pool(name="const", bufs=1))
        # weights: [128, KC*DS] (chunk k8 at free offset k8*DS)
        w_sb = []
        for t in range(3):
            w = const_pool.tile([128, KC * DS], f16, tag=f"w{t}", name=f"w{t}")
            nc.sync.dma_start(
                w[:].rearrange("p (kc m) -> p kc m", kc=KC),
                ins_W[t].rearrange("(kc p) m -> p kc m", p=128),
            )
            w_sb.append(w)
        b_sb = []
        for t in range(3):
            bt = const_pool.tile([128, 2], f32, tag=f"b{t}", name=f"b{t}")
            nc.sync.dma_start(bt[:], ins_B[t].rearrange("(c p) -> p c", p=128))
            b_sb.append(bt)
        id_sb = const_pool.tile([128, 128], f16, tag="ident")
        nc.sync.dma_start(id_sb[:], ident[:])
        idf_sb = const_pool.tile([65, 65], f32, tag="identf")
        nc.sync.dma_start(idf_sb[:], identf[0:65, 0:65])
        # warm the exp table while projections run
        scratch = const_pool.tile([128, 1], f32, tag="scratch")
        nc.scalar.activation(scratch[:], b_sb[0][:, 0:1], Exp)

        # persistent activations
        proj_pool = ctx.enter_context(tc.tile_pool(name="proj", bufs=1))
        qpT = [proj_pool.tile([128, S], f16, tag=f"qpT{m}", name=f"qpT{m}") for m in range(2)]
        kpT = [proj_pool.tile([128, S], f16, tag=f"kpT{m}", name=f"kpT{m}") for m in range(2)]
        # per head: [128, SEQC*65] fp16, col 64 of each 65-block is ones
        vpe = [proj_pool.tile([128, SEQC * 65], f16, tag=f"vpe{h}", name=f"vpe{h}") for h in range(HPC)]
        for h in range(HPC):
            nc.vector.memset(vpe[h][:, 64 :: 65], 1.0)

        out_sb = proj_pool.tile([128, SEQC * DS], f32, tag="out_sb")

        # ---------------- projections ----------------
        with (
            tc.tile_pool(name="pp", bufs=4, space="PSUM") as pp,
            tc.tile_pool(name="ptr", bufs=2, space="PSUM") as ptr,
            tc.tile_pool(name="rhs", bufs=9) as rhs_pool,
            tc.tile_pool(name="vtmp", bufs=3) as vtmp_pool,
        ):
            for t in range(3):
                rts = []
                for k8 in range(KC):
                    rt = rhs_pool.tile([128, S], f16, tag="rhs", name=f"rt{t}_{k8}")
                    nc.sync.dma_start(rt[:], ins_T[t][k8 * 128 : (k8 + 1) * 128, :])
                    rts.append(rt)
                for n in range(NQ):  # seq chunks of 512
                    ptiles = [pp.tile([128, 512], f32, tag="pp", name="pp") for _ in range(2)]
                    for k8 in range(KC):
                        for m in range(2):
                            nc.tensor.matmul(
                                ptiles[m][:],
                                w_sb[t][:, k8 * DS + m * 128 : k8 * DS + (m + 1) * 128],
                                rts[k8][:, n * 512 : (n + 1) * 512],
                                start=(k8 == 0),
                                stop=(k8 == KC - 1),
                            )
                    for m in range(2):
                        if t < 2:
                            dst = qpT[m] if t == 0 else kpT[m]
                            nc.vector.tensor_scalar_add(
                                dst[:, n * 512 : (n + 1) * 512],
                                ptiles[m][:],
                                b_sb[t][:, m : m + 1],
                            )
                        else:
                            vt = vtmp_pool.tile([128, 512], f16, tag="vtmp")
                            nc.vector.tensor_scalar_add(
                                vt[:], ptiles[m][:], b_sb[t][:, m : m + 1]
                            )
                            # transpose [128 dims, 128 seq] blocks -> vpe
                            for s4 in range(4):
                                c = n * 4 + s4  # global seq chunk
                                tp = ptr.tile([128, 128], f16, tag="ptr")
                                nc.tensor.transpose(
                                    tp[:],
                                    vt[:, s4 * 128 : (s4 + 1) * 128],
                                    id_sb[:],
                                )
                                for hh in range(2):
                                    h = m * 2 + hh
                                    nc.vector.tensor_copy(
                                        vpe[h][:, c * 65 : c * 65 + 64],
                                        tp[:, hh * 64 : hh * 64 + 64],
                                    )

        # ---------------- attention ----------------
        # Per chunk c: scores (4 MMs) -> exp -> AV of chunk c-1 (4 MMs, lag 1
        # so PE never waits on the current exp). All 4 AV accumulators stay
        # live across the chunk loop; out-transpose tiles share their slots.
        with (
            tc.tile_pool(name="sc", bufs=2, space="PSUM") as sc_pool,
            tc.tile_pool(name="av", bufs=4, space="PSUM") as av_pool,
            tc.tile_pool(name="epool", bufs=8) as e_pool,
            tc.tile_pool(name="avsb", bufs=2) as avsb_pool,
            tc.tile_pool(name="rcp", bufs=4) as rcp_pool,
        ):
            for h in range(HPC):
                m, off = h // 2, (h % 2) * 64

                avs = [av_pool.tile([65, 512], f32, tag="av", name=f"av{h}_{j}") for j in range(NQ)]

                def av_chunk(c, ec):
                    for j in range(NQ):
                        nc.tensor.matmul(
                            avs[j][:],
                            vpe[h][:, c * 65 : (c + 1) * 65],
                            ec[:, j * 512 : (j + 1) * 512],
                            start=(c == 0),
                            stop=(c == NK - 1),
                        )

                # half-chunk granularity: sc [128,1024] double-buffered so the
                # next chunk's scores never wait on exp; AV lags one half.
                prev = None  # (c, half, e-half) pending AV
                ehalves = {}

                def av_half(c, half, eh):
                    for j in (2 * half, 2 * half + 1):
                        nc.tensor.matmul(
                            avs[j][:],
                            vpe[h][:, c * 65 : (c + 1) * 65],
                            eh[:, (j - 2 * half) * 512 : (j - 2 * half + 1) * 512],
                            start=(c == 0),
                            stop=(c == NK - 1),
                        )

                for c in range(NK):
                    for half in range(2):
                        sc = sc_pool.tile([128, 1024], f32, tag="sc")
                        for j in (2 * half, 2 * half + 1):
                            nc.tensor.matmul(
                                sc[:, (j - 2 * half) * 512 : (j - 2 * half + 1) * 512],
                                kpT[m][off : off + 64, c * 128 : (c + 1) * 128],
                                qpT[m][off : off + 64, j * 512 : (j + 1) * 512],
                                start=True,
                                stop=True,
                            )
                        eh = e_pool.tile([128, 1024], f16, tag="e")
                        nc.scalar.activation(eh[:], sc[:], Exp, scale=0.125)
                        if prev is not None:
                            av_half(*prev)
                        prev = (c, half, eh)
                av_half(*prev)

                avsb = avsb_pool.tile([65, S], f32, tag="avsb")
                for j in range(NQ):
                    nc.vector.tensor_copy(avsb[:, j * 512 : (j + 1) * 512], avs[j][:])
                # transpose back + normalize
                for s in range(SEQC):
                    tp = av_pool.tile([128, 65], f32, tag="av", name=f"tr{h}_{s}")
                    nc.tensor.transpose(
                        tp[:],
                        avsb[:, s * 128 : (s + 1) * 128],
                        idf_sb[:],
                    )
                    rc = rcp_pool.tile([128, 1], f32, tag="rcp")
                    nc.vector.reciprocal(rc[:], tp[:, 64:65])
                    nc.vector.tensor_scalar_mul(
                        out_sb[:, s * DS + h * 64 : s * DS + (h + 1) * 64],
                        tp[:, 0:64],
                        rc[:, 0:1],
                    )
                # per-head output store
                nc.sync.dma_start(
                    out[:, h * 64 : (h + 1) * 64].rearrange("(c p) d -> p c d", p=128),
                    out_sb[:]
                    .rearrange("p (c hd) -> p c hd", c=SEQC)[
                        :, :, h * 64 : (h + 1) * 64
                    ],
                )

    nc.compile()
    return nc


def _get_nc():
    if "nc" not in _CACHE:
        _CACHE["nc"] = _build()
    return _CACHE["nc"]


def _in_maps(q, k, v, Wq, bq, Wk, bk, Wv, bv):
    q, k, v = (np.asarray(x, np.float32) for x in (q, k, v))
    Wq, Wk, Wv = (np.asarray(x, np.float32) for x in (Wq, Wk, Wv))
    bq, bk, bv = (np.asarray(x, np.float32) for x in (bq, bk, bv))
    ident = np.eye(128, dtype=np.float32)
    maps = []
    for c in range(NCORES):
        b, g = divmod(c, NCORES // B)
        sl = slice(g * DS, (g + 1) * DS)
        maps.append(
            {
                "qT": np.ascontiguousarray(q[b].T).astype(ml_dtypes.bfloat16),
                "kT": np.ascontiguousarray(k[b].T).astype(ml_dtypes.bfloat16),
                "vT": np.ascontiguousarray(v[b].T).astype(ml_dtypes.bfloat16),
                "wqT": np.ascontiguousarray(Wq[sl].T).astype(ml_dtypes.bfloat16),
                "wkT": np.ascontiguousarray(Wk[sl].T).astype(ml_dtypes.bfloat16),
                "wvT": np.ascontiguousarray(Wv[sl].T).astype(ml_dtypes.bfloat16),
                "bq": np.ascontiguousarray(bq[sl]),
                "bk": np.ascontiguousarray(bk[sl]),
                "bv": np.ascontiguousarray(bv[sl]),
                "ident": ident.astype(ml_dtypes.bfloat16),
                "identf": ident,
            }
        )
    return maps


def _assemble(results):
    out = np.empty((B, S, D), np.float32)
    for c in range(NCORES):
        b, g = divmod(c, NCORES // B)
        out[b, :, g * DS : (g + 1) * DS] = results[c]["out"]
    return out


def kernel(q, k, v, Wq, bq, Wk, bk, Wv, bv):
    from concourse.bass_utils import run_bass_kernel_spmd

    nc = _get_nc()
    maps = _in_maps(q, k, v, Wq, bq, Wk, bk, Wv, bv)
    res = run_bass_kernel_spmd(nc, maps, list(range(NCORES)))
    return _assemble(res.results)


def run_traced(q, k, v, Wq, bq, Wk, bk, Wv, bv, trace_cores=None):
    """Like kernel() but with NTFF profiling; returns (out, exec_time_ns, results)."""
    import sys, types

    if "antenv.axon_hooks" not in sys.modules:
        mod = types.ModuleType("antenv.axon_hooks")
        _hook = [None]
        mod.set_axon_ntff_profile_hook = lambda h: _hook.__setitem__(0, h)
        mod.get_axon_ntff_profile_hook = lambda: _hook[0]
        sys.modules["antenv.axon_hooks"] = mod
        import antenv

        antenv.axon_hooks = mod
        try:
            from trn_agent_boot.trn_boot import _ntff_profile_via_ctypes

            mod.set_axon_ntff_profile_hook(
                _ntff_profile_via_ctypes("/opt/axon/libaxon_pjrt.so")
            )
        except Exception as e:
            print("ntff hook setup failed:", e)
    import concourse.bass_utils as bass_utils

    bass_utils.upload_artifacts = lambda tmpdir: tmpdir

    nc = _get_nc()
    maps = _in_maps(q, k, v, Wq, bq, Wk, bk, Wv, bv)
    res = bass_utils.run_bass_kernel_spmd(
        nc, maps, list(range(NCORES)), trace=True, trace_cores=trace_cores
    )
    return _assemble(res.results), res.exec_time_ns, res
